# revision 10
# baseline (speedup 1.0000x reference)
"""VQ codebook assignment (ApplyKmeans) on 8 Trainium2 NeuronCores.

tokens[n] = argmin_k ||x_n - c_k||^2 = argmax_k (x_n.c_k - Cnorm_k/2)
(||x_n||^2 is constant per row). Data-parallel: x sharded along N
across 8 cores, C/Cnorm replicated.

Per core (16384 rows, 128 row-tiles of 128 rows), v4 schedule -- PE is
the only critical engine; every other engine runs below the 2080ns/pair
PE budget, and the startup is arrival-order-matched at 2-chunk
granularity:
  - host pre-tiles x^T so each [128d, 128n] stationary tile is
    contiguous (fp16: halves HBM traffic; PSUM accumulates fp32;
    52/131072 argmin flips vs the fp32 reference, rel err 0.0144)
  - PSUM is 4 pair-tiles [128, 1024] f32 (= 2 banks each); row-tiles
    2p / 2p+1 accumulate into the halves at col 0 / 512 so each matmul
    output sits fully inside one bank
  - per tile: 8 accumulating matmuls (x^T chunk stationary, C chunk
    moving) on top of a bias pre-load, so PSUM holds
    val = x.C - Cnorm/2
  - bias pre-load: tiles 0-7 matmul the bias in (ones x [-Cnorm/2
    hi/lo] rows, start=True), which also covers the PSUM has_written
    bits; pairs 4+ get ONE ScalarE ACTIVATE per pair writing both
    halves ([128,2,300] strided view), emitted a pair ahead of the PE
  - per pair, ONE ScalarE ACTIVATE evicts both halves PSUM->SBUF
    (fp32, exact); DVE max8 + max_index run on the SBUF copy
    (58-cycle access vs 120 for PSUM), dropping DVE to ~1.8us/pair
    vs the 2.08us PE budget (v1's DVE-on-PSUM was co-critical with
    the PE and every jitter stalled both)
  - startup (measured: first DMA data ~9.0us after a ~7.5us framework
    preamble, early aggregate ~300GB/s): DMA_DIRECT2D issues cost
    ~670ns each on the in-order queues, and a transfer's completion
    sem only fires when the whole transfer lands, so groups 0-2 go
    out as 4 two-chunk batches each (progressive unlock) while cons
    is split [bias+ones, c0-1, c2-4, c5-7] on the parallel scalar
    ring; tiles 0-11 (groups 0-2) run j-major -- all 4 tiles' matmuls
    for chunk j back-to-back -- so every arriving batch unlocks ~1us
    of PE work; groups 3+ are monolithic transfers consumed
    tile-major by the steady pair loop (DMA is ahead by then)
  - prologue engine-order care: each pair's evict is emitted
    immediately after that pair's last matmul, and the pair 4/5/6
    bias writes are interleaved between the right evicts on the
    in-order scalar queue, with PSUM slots allocated so each bias
    write's WAR dependency (evict of the pair 4 back) is long done --
    in v3 the pair-4 bias sat behind all four prologue evicts and
    stalled the PE 3us (long enough to re-throttle HAM)
  - dep-free warmup matmuls over a memset tile ramp the PE p-state
    during the startup DMA wait; a tiny early ACTIVATE hoists the
    ~1.3us ACT_TABLE_LOAD into the same window
  - tokens compacted on the otherwise-idle Pool engine and streamed
    out in 16-tile blocks through t=112, then 8/6/2; flush DMA issues
    ride the idle sync queue, deferred one pair so they never park
    ahead of an x-group prefetch issue
  - the last pair skips the eviction: DVE runs on PSUM directly, so
    tile 126's argmax overlaps tile 127's matmuls and the tail chain
    after the last matmul is just max8+find+match+cast+DMA (~1.7us)

Row interleaving: row-tile t holds rows {p*128 + t}, so the token
buffer [p, t] DMAs out contiguously in original row order.

Walrus only lowers one sync wait per instruction; _hoist_excess_waits
moves Tile's extra waits onto same-engine no-ops at the same program
point. x loads share the sync HWDGE ring (same-ring transfers complete
in order, so prefetch can't starve urgent loads); constants ride the
scalar ring.

Profiling notes: engine clock varies run to run (2.4 vs ~2.0 GHz
throttle episodes: check MATMUL median duration ~290ns vs ~348ns in
the profile before comparing timings). Steady-state PE floor is
~127.2ns/matmul; measured mid-stream cadence 128.5ns/MM. 1024 matmuls
=> ~131.6us + ~10us startup + ~4us tail.
"""
import os
import sys

import numpy as np

if "/opt/trn_rl_repo" not in sys.path:
    sys.path.insert(0, "/opt/trn_rl_repo")

import concourse.bass as bass
import concourse.mybir as mybir
import concourse.tile_sem_assignment as _tsa
from concourse.bass_utils import run_bass_kernel_spmd
from concourse.tile import TileContext

_tsa.NUM_HWDGE_SEMS = int(os.environ.get("KM_HW_LANES", "8"))

_orig_assign_tick = _tsa.TileClockTick._assign_tick


def _assign_tick_lanepools(self, inst):
    try:
        if isinstance(inst, _tsa.DMAInst) and inst.engine != mybir.EngineType.Pool:
            if not hasattr(self, "_lane_ctr"):
                self._lane_ctr = {}
            eng = inst.engine
            n = _tsa.NUM_HWDGE_SEMS
            half = max(1, n // 2)
            pool = (
                list(range(0, half))
                if eng == mybir.EngineType.Activation
                else list(range(half, n))
            )
            c = self._lane_ctr.get(eng, 0)
            self.next_hw_dma_idx = pool[c % len(pool)]
            self._lane_ctr[eng] = c + 1
    except Exception:
        pass
    return _orig_assign_tick(self, inst)


_tsa.TileClockTick._assign_tick = _assign_tick_lanepools

P = 128
D = 1024
K = 300
NCORES = 8
ROWS = 16384
TILES = ROWS // P
GROUPS = 32
TPG = TILES // GROUPS
DCH = D // P
PAIRS = TILES // 2
HALF = 512  # fp32 elements per PSUM bank
JM_GROUPS = 3  # groups 0..2 (tiles 0-11) run j-major on batched loads

F16 = mybir.dt.float16
F32 = mybir.dt.float32
I32 = mybir.dt.int32
U32 = mybir.dt.uint32

LAST_RESULT = None


def _ensure_ntff_hook():
    try:
        from antenv.axon_hooks import get_axon_ntff_profile_hook  # noqa: F401

        return
    except ImportError:
        pass
    import types

    import antenv

    try:
        from trn_agent_boot.trn_boot import _ntff_profile_via_ctypes
    except ImportError:
        return
    mod = types.ModuleType("antenv.axon_hooks")
    _hook = [None]
    mod.set_axon_ntff_profile_hook = lambda h: _hook.__setitem__(0, h)
    mod.get_axon_ntff_profile_hook = lambda: _hook[0]
    sys.modules["antenv.axon_hooks"] = mod
    antenv.axon_hooks = mod
    so = "/opt/axon/libaxon_pjrt.so"
    if os.path.exists(so):
        mod.set_axon_ntff_profile_hook(_ntff_profile_via_ctypes(so))


def build_nc() -> bass.Bass:
    nc = bass.Bass()

    xg = nc.declare_dram_parameter("xg", [GROUPS, P, DCH * TPG * P], F16, isOutput=False)
    cons = nc.declare_dram_parameter("cons", [P, DCH * K + K + P], F16, isOutput=False)
    biasf = nc.declare_dram_parameter("biasf", [P, 2 * K], F32, isOutput=False)
    out = nc.declare_dram_parameter("out", [P, TILES], I32, isOutput=True)

    FLUSH = [16, 32, 48, 64, 80, 96, 112, 120, 126, 128]

    with TileContext(nc) as tc:
        with (
            tc.tile_pool(name="const", bufs=1) as constp,
            tc.tile_pool(name="warm", bufs=1) as warmp,
            tc.tile_pool(name="xp0", bufs=4 * JM_GROUPS) as xp0,
            tc.tile_pool(name="xp", bufs=4) as xp,
            tc.tile_pool(name="mx", bufs=8) as mxp,
            tc.tile_pool(name="val", bufs=4) as valp,
            tc.tile_pool(name="psum", bufs=4, space="PSUM") as psp,
            tc.tile_pool(name="outp", bufs=1) as outp,
        ):
            # cons splits ordered to match consumption: bias+ones first
            # (for the tiles-0-7 bias matmuls), then C chunks in j order
            # rate-matched against the x batches on the other ring
            # host layout [bias K | ones P | c0..c7]: the first piece
            # (bias+ones+c0+c1, 264KB) lands as ONE completion sem so
            # the bias matmuls and chunk-0/1 matmuls gate together at
            # ~11.5us instead of chaining two transfer sems
            CB = K + P  # offset of c0
            cons_t = constp.tile([P, DCH * K + K + P], F16)
            nc.scalar.dma_start(
                out=cons_t[:, : CB + 2 * K], in_=cons[:, : CB + 2 * K]
            )
            nc.scalar.dma_start(
                out=cons_t[:, CB + 2 * K : CB + 5 * K],
                in_=cons[:, CB + 2 * K : CB + 5 * K],
            )
            nc.scalar.dma_start(
                out=cons_t[:, CB + 5 * K :], in_=cons[:, CB + 5 * K :]
            )
            ctiles = [
                cons_t[:, CB + j * K : CB + (j + 1) * K] for j in range(DCH)
            ]
            btile = cons_t[:, :K]
            otile = cons_t[:, K:CB]
            bft = constp.tile([P, 2, K], F32)
            nc.scalar.dma_start(
                out=bft[:], in_=biasf[:].rearrange("p (b k) -> p b k", b=2)
            )

            # PE warmup: dep-free matmuls over a memset tile, covering
            # both halves of all 4 PSUM pair tiles. They execute during
            # the startup DMA wait, ramping the PE out of its low
            # p-state, and their start=True writes cover the PSUM
            # has_written bits alongside the tiles-0-7 bias matmuls.
            warm = warmp.tile([P, K], F16)
            nc.gpsimd.memset(warm[:], 0.0)
            # tiny dummy ACTIVATE: hoists the ~1.3us ACT_TABLE_LOAD
            # into the startup DMA wait (otherwise it fires right
            # before the first evict)
            wsc = warmp.tile([P, 8], F32, name="wsc")
            nc.scalar.copy(out=wsc[:1, :], in_=warm[:1, :8])
            for w in range(8):
                wps = psp.tile([P, 2 * HALF], F32, name="ps")
                for h in range(2):
                    nc.tensor.matmul(
                        wps[:, h * HALF : h * HALF + K],
                        lhsT=warm[:, :P],
                        rhs=warm[:],
                        start=True,
                        stop=True,
                    )

            # x loads: groups 0-2 in 4 two-chunk batches each (few
            # ~670ns issue slots, progressive completion sems on the
            # in-order ring); groups 3+ monolithic
            xbat = {}
            gbufs = {}

            def issue_group_batched(g):
                bufs = []
                for i in range(4):
                    cb = xp0.tile([P, 2, TPG, P], F16, name="xb")
                    nc.sync.dma_start(
                        out=cb[:],
                        in_=xg[
                            g, :, 2 * i * TPG * P : 2 * (i + 1) * TPG * P
                        ].rearrange("p (j t q) -> p j t q", j=2, t=TPG),
                    )
                    bufs.append(cb)
                xbat[g] = bufs

            def issue_group(g):
                xbuf = xp.tile([P, DCH, TPG, P], F16, name="xgrp")
                nc.sync.dma_start(
                    out=xbuf[:],
                    in_=xg[g].rearrange("p (j t q) -> p j t q", j=DCH, t=TPG),
                )
                gbufs[g] = xbuf

            for g in range(JM_GROUPS):
                issue_group_batched(g)
            issue_group(3)
            issue_group(4)
            issue_group(5)

            def chunk_of(g):
                if g < JM_GROUPS:
                    return lambda j, tl, bufs=xbat[g]: bufs[j // 2][:, j % 2, tl, :]
                return lambda j, tl, xb=gbufs[g]: xb[:, j, tl, :]

            idxbuf = outp.tile([P, TILES, 8], U32)
            tokbuf = outp.tile([P, TILES], I32)

            def emit_flush(lo, hi):
                # token compaction on the otherwise-idle Pool engine;
                # the DMA issue rides the (idle) sync queue
                nc.gpsimd.tensor_copy(
                    out=tokbuf[:, lo:hi], in_=idxbuf[:, lo:hi, 0]
                )
                nc.sync.dma_start(out=out[:, lo:hi], in_=tokbuf[:, lo:hi])

            def pair_view(ps):
                # [128, 2, 300] strided view of the two bank halves
                return ps[:].rearrange("p (b q) -> p b q", b=2)[:, :, :K]

            def dst(ps, h):
                return ps[:, h * HALF : h * HALF + K]

            def dve_tile(t, values):
                mx = mxp.tile([P, 8], F32)
                nc.vector.max(out=mx[:], in_=values)
                nc.vector.max_index(
                    out=idxbuf[:, t, :], in_max=mx[:], in_values=values
                )

            ps_pairs = {}

            def alloc_pair(pr):
                ps_pairs[pr] = psp.tile([P, 2 * HALF], F32, name="ps")
                return ps_pairs[pr]

            def bias_write(pr):
                nc.scalar.copy(out=pair_view(alloc_pair(pr)), in_=bft[:])

            def evict_dve(pr):
                val = valp.tile([P, 2, K], F32, name="val")
                nc.scalar.copy(out=val[:], in_=pair_view(ps_pairs[pr]))
                for h in range(2):
                    dve_tile(2 * pr + h, val[:, h, :])

            def jmajor_group(g, bias_mm, mid_scalar=None, post_scalar=None):
                # all 4 tiles' matmuls for chunk j back-to-back, so
                # every arriving 2-chunk batch unlocks ~1us of PE work;
                # evicts inline right after each pair's last matmul
                p0, p1 = 2 * g, 2 * g + 1
                ps0, ps1 = ps_pairs[p0], ps_pairs[p1]
                chunk = chunk_of(g)
                if bias_mm:
                    for tl in range(4):
                        nc.tensor.matmul(
                            dst((ps0, ps1)[tl // 2], tl % 2),
                            lhsT=otile[:], rhs=btile[:],
                            start=True, stop=False,
                            skip_group_check=True,
                        )
                for j in range(DCH):
                    last = j == DCH - 1
                    for tl in (0, 1):
                        nc.tensor.matmul(
                            dst(ps0, tl),
                            lhsT=chunk(j, tl), rhs=ctiles[j][:],
                            start=False, stop=last,
                            skip_group_check=True,
                        )
                    if last:
                        # bias writes go BEFORE the adjacent evict on
                        # the in-order scalar queue: their WAR dep (an
                        # earlier pair's evict) is already done, while
                        # this evict's sem may fire much later
                        if mid_scalar is not None:
                            mid_scalar()
                        evict_dve(p0)
                    for tl in (2, 3):
                        nc.tensor.matmul(
                            dst(ps1, tl - 2),
                            lhsT=chunk(j, tl), rhs=ctiles[j][:],
                            start=False, stop=last,
                            skip_group_check=True,
                        )
                    if last:
                        if post_scalar is not None:
                            post_scalar()
                        evict_dve(p1)

            # ---- prologue: tiles 0-11 (groups 0-2, pairs 0-5)
            # PSUM slot order: pairs 0,1,2,3 -> slots 0-3; the pair
            # 4/5/6 bias writes then land on slots 0/1/2 whose evicts
            # (pairs 0/1/2) are already done when they reach the queue
            # head -- scalar queue: e0 b4 e1 | b5 e2 e3 | b6 e4 e5
            for pr in range(4):
                alloc_pair(pr)
            jmajor_group(0, bias_mm=True, post_scalar=lambda: bias_write(4))
            jmajor_group(1, bias_mm=True, mid_scalar=lambda: bias_write(5))
            jmajor_group(2, bias_mm=False, mid_scalar=lambda: bias_write(6))

            # ---- pairs 6-63: steady state
            pending = None
            for pr in range(2 * JM_GROUPS, PAIRS):
                t0 = 2 * pr
                g = t0 // TPG
                chunk = chunk_of(g)
                if t0 % TPG == 0 and g + 3 < GROUPS and (g + 3) not in gbufs:
                    # prefetch three groups ahead at each group start
                    # (xp bufs=4: active + 3 in flight)
                    issue_group(g + 3)
                ps = ps_pairs[pr]
                if pr + 1 < PAIRS:
                    # bias write for the NEXT pair, one pair ahead of
                    # the PE so the in-order scalar queue never stalls
                    # the matmul stream
                    bias_write(pr + 1)
                # deferred flush: emitted after the next pair's bias
                # copy so the token-DMA issue never parks ahead of an
                # x prefetch issue on the sync queue
                if pending is not None:
                    emit_flush(*pending)
                    pending = None

                for h in range(2):
                    t = t0 + h
                    for j in range(DCH):
                        nc.tensor.matmul(
                            dst(ps, h),
                            lhsT=chunk(j, t % TPG),
                            rhs=ctiles[j][:],
                            start=False,
                            stop=(j == DCH - 1),
                            skip_group_check=True,
                        )

                if pr == PAIRS - 1:
                    # tail: skip the evict, DVE reads PSUM directly so
                    # tile 126's argmax overlaps tile 127's matmuls
                    for h in range(2):
                        dve_tile(t0 + h, dst(ps, h))
                else:
                    evict_dve(pr)
                for h in range(2):
                    t = t0 + h
                    if (t + 1) in FLUSH:
                        s = FLUSH[FLUSH.index(t + 1) - 1] if (t + 1) != FLUSH[0] else 0
                        if t + 1 == TILES:
                            emit_flush(s, t + 1)
                        else:
                            pending = (s, t + 1)

    _hoist_excess_waits(nc)
    return nc


def _hoist_excess_waits(nc: bass.Bass, max_waits: int = 1):
    n = 0
    for f in nc.m.functions:
        for blk in f.blocks:
            insts = blk.instructions
            i = 0
            while i < len(insts):
                inst = insts[i]
                si = inst.sync_info
                if si and si.on_wait and len(si.on_wait) > max_waits:
                    waits = list(si.on_wait)
                    si.on_wait = waits[-max_waits:]
                    inst.sync_info = si
                    pre = []
                    for j in range(0, len(waits) - max_waits, max_waits):
                        nd = mybir.InstNoOp(name=f"I-wsplit{n}", ins=[], outs=[])
                        n += 1
                        nd.engine = inst.engine
                        nsi = type(si)(
                            on_wait=waits[j : j + max_waits], on_update=[]
                        )
                        nd.sync_info = nsi
                        try:
                            nc.register_instruction(nd, overwrite=True)
                        except Exception:
                            pass
                        pre.append(nd)
                    for k, nd in enumerate(pre):
                        insts.insert(i + k, nd)
                    i += len(pre)
                i += 1


def make_in_maps(x, C, Cnorm):
    x16 = x.astype(np.float16)
    C16 = C.astype(np.float16).reshape(DCH, P, K)

    bz = (-0.5 * Cnorm.reshape(K)).astype(np.float32)
    bh = bz.astype(np.float16)
    bl = (bz - bh.astype(np.float32)).astype(np.float16)

    cons = np.zeros((P, DCH * K + K + P), np.float16)
    cons[0, :K] = bh
    cons[1, :K] = bl
    cons[0:2, K : K + P] = 1.0
    cons[:, K + P :] = C16.transpose(1, 0, 2).reshape(P, DCH * K)

    brow = bh.astype(np.float32) + bl.astype(np.float32)
    biasf = np.broadcast_to(
        np.concatenate([brow, brow]), (P, 2 * K)
    ).copy()

    in_maps = []
    for c in range(NCORES):
        xs = x16[c * ROWS : (c + 1) * ROWS]
        xr = xs.reshape(P, GROUPS, TPG, DCH, P)
        xgc = np.ascontiguousarray(xr.transpose(1, 4, 3, 2, 0))
        in_maps.append(
            {
                "xg": xgc.reshape(GROUPS, P, DCH * TPG * P),
                "cons": cons,
                "biasf": biasf,
            }
        )
    return in_maps


_NC_CACHE = {}


def kernel(x, C, Cnorm, b, t):
    global LAST_RESULT
    x = np.asarray(x)
    C = np.asarray(C)
    Cnorm = np.asarray(Cnorm)

    key = 0
    if key not in _NC_CACHE:
        _NC_CACHE[key] = build_nc()
    nc = _NC_CACHE[key]

    in_maps = make_in_maps(x, C, Cnorm)
    trace = bool(int(os.environ.get("KM_TRACE", "0")))
    if trace:
        _ensure_ntff_hook()
    res = run_bass_kernel_spmd(
        nc, in_maps, core_ids=list(range(NCORES)), trace=trace
    )
    LAST_RESULT = res

    shards = [res.results[c]["out"].reshape(-1) for c in range(NCORES)]
    tokens = np.concatenate(shards).astype(np.int32)
    return tokens.reshape(int(b), int(t))


# revision 11
# speedup vs baseline: 1.0140x; 1.0140x over previous
"""VQ codebook assignment (ApplyKmeans) on 8 Trainium2 NeuronCores.

tokens[n] = argmin_k ||x_n - c_k||^2 = argmax_k (x_n.c_k - Cnorm_k/2)
(||x_n||^2 is constant per row). Data-parallel: x sharded along N
across 8 cores, C/Cnorm replicated.

Per core (16384 rows, 128 row-tiles of 128 rows), v4 schedule -- PE is
the only critical engine; every other engine runs below the 2080ns/pair
PE budget, and the startup is arrival-order-matched at 2-chunk
granularity:
  - host pre-tiles x^T so each [128d, 128n] stationary tile is
    contiguous (fp16: halves HBM traffic; PSUM accumulates fp32;
    52/131072 argmin flips vs the fp32 reference, rel err 0.0144)
  - PSUM is 4 pair-tiles [128, 1024] f32 (= 2 banks each); row-tiles
    2p / 2p+1 accumulate into the halves at col 0 / 512 so each matmul
    output sits fully inside one bank
  - per tile: 8 accumulating matmuls (x^T chunk stationary, C chunk
    moving) on top of a bias pre-load, so PSUM holds
    val = x.C - Cnorm/2
  - bias pre-load: tiles 0-7 matmul the bias in (ones x [-Cnorm/2
    hi/lo] rows, start=True), which also covers the PSUM has_written
    bits; pairs 4+ get ONE ScalarE ACTIVATE per pair writing both
    halves ([128,2,300] strided view), emitted a pair ahead of the PE
  - per pair, ONE ScalarE ACTIVATE evicts both halves PSUM->SBUF
    (fp32, exact); DVE max8 + max_index run on the SBUF copy
    (58-cycle access vs 120 for PSUM), dropping DVE to ~1.8us/pair
    vs the 2.08us PE budget (v1's DVE-on-PSUM was co-critical with
    the PE and every jitter stalled both)
  - startup (measured: first DMA data ~9.0us after a ~7.5us framework
    preamble, early aggregate ~300GB/s): DMA_DIRECT2D issues cost
    ~670ns each on the in-order queues, and a transfer's completion
    sem only fires when the whole transfer lands, so groups 0-2 go
    out as 4 two-chunk batches each (progressive unlock) while cons
    is split [bias+ones, c0-1, c2-4, c5-7] on the parallel scalar
    ring; tiles 0-11 (groups 0-2) run j-major -- all 4 tiles' matmuls
    for chunk j back-to-back -- so every arriving batch unlocks ~1us
    of PE work; groups 3+ are monolithic transfers consumed
    tile-major by the steady pair loop (DMA is ahead by then)
  - prologue engine-order care: each pair's evict is emitted
    immediately after that pair's last matmul, and the pair 4/5/6
    bias writes are interleaved between the right evicts on the
    in-order scalar queue, with PSUM slots allocated so each bias
    write's WAR dependency (evict of the pair 4 back) is long done --
    in v3 the pair-4 bias sat behind all four prologue evicts and
    stalled the PE 3us (long enough to re-throttle HAM)
  - dep-free warmup matmuls over a memset tile ramp the PE p-state
    during the startup DMA wait; a tiny early ACTIVATE hoists the
    ~1.3us ACT_TABLE_LOAD into the same window
  - tokens compacted on the otherwise-idle Pool engine and streamed
    out in 16-tile blocks through t=112, then 8/6/2; flush DMA issues
    ride the idle sync queue, deferred one pair so they never park
    ahead of an x-group prefetch issue
  - the last pair skips the eviction: DVE runs on PSUM directly, so
    tile 126's argmax overlaps tile 127's matmuls and the tail chain
    after the last matmul is just max8+find+match+cast+DMA (~1.7us)

Row interleaving: row-tile t holds rows {p*128 + t}, so the token
buffer [p, t] DMAs out contiguously in original row order.

Walrus only lowers one sync wait per instruction; _hoist_excess_waits
moves Tile's extra waits onto same-engine no-ops at the same program
point. x loads share the sync HWDGE ring (same-ring transfers complete
in order, so prefetch can't starve urgent loads); constants ride the
scalar ring.

Profiling notes: engine clock varies run to run (2.4 vs ~2.0 GHz
throttle episodes: check MATMUL median duration ~290ns vs ~348ns in
the profile before comparing timings). Steady-state PE floor is
~127.2ns/matmul; measured mid-stream cadence 128.5ns/MM. 1024 matmuls
=> ~131.6us + ~10us startup + ~4us tail.
"""
import os
import sys

import numpy as np

if "/opt/trn_rl_repo" not in sys.path:
    sys.path.insert(0, "/opt/trn_rl_repo")

import concourse.bass as bass
import concourse.mybir as mybir
import concourse.tile_sem_assignment as _tsa
from concourse.bass_utils import run_bass_kernel_spmd
from concourse.tile import TileContext

_tsa.NUM_HWDGE_SEMS = int(os.environ.get("KM_HW_LANES", "8"))

_orig_assign_tick = _tsa.TileClockTick._assign_tick


def _assign_tick_lanepools(self, inst):
    try:
        if isinstance(inst, _tsa.DMAInst) and inst.engine != mybir.EngineType.Pool:
            if not hasattr(self, "_lane_ctr"):
                self._lane_ctr = {}
            eng = inst.engine
            n = _tsa.NUM_HWDGE_SEMS
            half = max(1, n // 2)
            pool = (
                list(range(0, half))
                if eng == mybir.EngineType.Activation
                else list(range(half, n))
            )
            c = self._lane_ctr.get(eng, 0)
            self.next_hw_dma_idx = pool[c % len(pool)]
            self._lane_ctr[eng] = c + 1
    except Exception:
        pass
    return _orig_assign_tick(self, inst)


_tsa.TileClockTick._assign_tick = _assign_tick_lanepools

P = 128
D = 1024
K = 300
NCORES = 8
ROWS = 16384
TILES = ROWS // P
GROUPS = 32
TPG = TILES // GROUPS
DCH = D // P
PAIRS = TILES // 2
HALF = 512  # fp32 elements per PSUM bank
JM_GROUPS = 3  # groups 0..2 (tiles 0-11) run j-major on batched loads

F16 = mybir.dt.float16
F32 = mybir.dt.float32
I32 = mybir.dt.int32
U32 = mybir.dt.uint32

LAST_RESULT = None


def _ensure_ntff_hook():
    try:
        from antenv.axon_hooks import get_axon_ntff_profile_hook  # noqa: F401

        return
    except ImportError:
        pass
    import types

    import antenv

    try:
        from trn_agent_boot.trn_boot import _ntff_profile_via_ctypes
    except ImportError:
        return
    mod = types.ModuleType("antenv.axon_hooks")
    _hook = [None]
    mod.set_axon_ntff_profile_hook = lambda h: _hook.__setitem__(0, h)
    mod.get_axon_ntff_profile_hook = lambda: _hook[0]
    sys.modules["antenv.axon_hooks"] = mod
    antenv.axon_hooks = mod
    so = "/opt/axon/libaxon_pjrt.so"
    if os.path.exists(so):
        mod.set_axon_ntff_profile_hook(_ntff_profile_via_ctypes(so))


def build_nc() -> bass.Bass:
    nc = bass.Bass()

    xg = nc.declare_dram_parameter("xg", [GROUPS, P, DCH * TPG * P], F16, isOutput=False)
    cons = nc.declare_dram_parameter("cons", [P, DCH * K + K + P], F16, isOutput=False)
    out = nc.declare_dram_parameter("out", [P, TILES], I32, isOutput=True)

    FLUSH = [16, 32, 48, 64, 80, 96, 112, 120, 127, 128]

    with TileContext(nc) as tc:
        with (
            tc.tile_pool(name="const", bufs=1) as constp,
            tc.tile_pool(name="warm", bufs=1) as warmp,
            tc.tile_pool(name="xp0", bufs=4 * JM_GROUPS) as xp0,
            tc.tile_pool(name="xp", bufs=4) as xp,
            tc.tile_pool(name="mx", bufs=8) as mxp,
            tc.tile_pool(name="val", bufs=4) as valp,
            tc.tile_pool(name="psum", bufs=4, space="PSUM") as psp,
            tc.tile_pool(name="outp", bufs=1) as outp,
        ):
            # cons splits ordered to match consumption: bias+ones first
            # (for the tiles-0-7 bias matmuls), then C chunks in j order
            # rate-matched against the x batches on the other ring
            # host layout [bias K | ones P | c0..c7]: the first piece
            # (bias+ones+c0+c1, 264KB) lands as ONE completion sem so
            # the bias matmuls and chunk-0/1 matmuls gate together at
            # ~11.5us instead of chaining two transfer sems
            CB = K + P  # offset of c0
            cons_t = constp.tile([P, DCH * K + K + P], F16)
            nc.scalar.dma_start(
                out=cons_t[:, : CB + 2 * K], in_=cons[:, : CB + 2 * K]
            )
            nc.scalar.dma_start(
                out=cons_t[:, CB + 2 * K : CB + 5 * K],
                in_=cons[:, CB + 2 * K : CB + 5 * K],
            )
            nc.scalar.dma_start(
                out=cons_t[:, CB + 5 * K :], in_=cons[:, CB + 5 * K :]
            )
            ctiles = [
                cons_t[:, CB + j * K : CB + (j + 1) * K] for j in range(DCH)
            ]
            btile = cons_t[:, :K]
            otile = cons_t[:, K:CB]
            # fp32 bias table built on-chip instead of a 307KB DMA on
            # the bandwidth-critical front: one extra bias matmul
            # broadcasts ones x [bh; bl] into a PSUM scratch region
            # (fp32-exact bh+bl), then two startup ACTIVATEs copy it
            # into both halves of the SBUF table
            bft = constp.tile([P, 2, K], F32)

            # PE warmup: dep-free matmuls over a memset tile, covering
            # both halves of all 4 PSUM pair tiles. They execute during
            # the startup DMA wait, ramping the PE out of its low
            # p-state, and their start=True writes cover the PSUM
            # has_written bits alongside the tiles-0-7 bias matmuls.
            warm = warmp.tile([P, K], F16)
            nc.gpsimd.memset(warm[:], 0.0)
            # tiny dummy ACTIVATE: hoists the ~1.3us ACT_TABLE_LOAD
            # into the startup DMA wait (otherwise it fires right
            # before the first evict)
            wsc = warmp.tile([P, 8], F32, name="wsc")
            nc.scalar.copy(out=wsc[:1, :], in_=warm[:1, :8])
            for w in range(6):
                wps = psp.tile([P, 2 * HALF], F32, name="ps")
                for h in range(2):
                    nc.tensor.matmul(
                        wps[:, h * HALF : h * HALF + K],
                        lhsT=warm[:, :P],
                        rhs=warm[:],
                        start=True,
                        stop=True,
                    )
                if w == 5:
                    # bias broadcast into the scratch region, evicted
                    # twice into the bias table during the DMA wait
                    nc.tensor.matmul(
                        wps[:, :K], lhsT=otile[:], rhs=btile[:],
                        start=True, stop=True,
                    )
                    for b in range(2):
                        nc.scalar.copy(out=bft[:, b, :], in_=wps[:, :K])

            # x loads: groups 0-2 in 4 two-chunk batches each (few
            # ~670ns issue slots, progressive completion sems on the
            # in-order ring); groups 3+ monolithic
            xbat = {}
            gbufs = {}

            def issue_group_batched(g):
                bufs = []
                for i in range(4):
                    cb = xp0.tile([P, 2, TPG, P], F16, name="xb")
                    nc.sync.dma_start(
                        out=cb[:],
                        in_=xg[
                            g, :, 2 * i * TPG * P : 2 * (i + 1) * TPG * P
                        ].rearrange("p (j t q) -> p j t q", j=2, t=TPG),
                    )
                    bufs.append(cb)
                xbat[g] = bufs

            def issue_group(g):
                xbuf = xp.tile([P, DCH, TPG, P], F16, name="xgrp")
                nc.sync.dma_start(
                    out=xbuf[:],
                    in_=xg[g].rearrange("p (j t q) -> p j t q", j=DCH, t=TPG),
                )
                gbufs[g] = xbuf

            for g in range(JM_GROUPS):
                issue_group_batched(g)
            issue_group(3)
            issue_group(4)
            issue_group(5)

            def chunk_of(g):
                if g < JM_GROUPS:
                    return lambda j, tl, bufs=xbat[g]: bufs[j // 2][:, j % 2, tl, :]
                return lambda j, tl, xb=gbufs[g]: xb[:, j, tl, :]

            idxbuf = outp.tile([P, TILES, 8], U32)
            tokbuf = outp.tile([P, TILES], I32)

            def emit_flush(lo, hi):
                # token compaction on the otherwise-idle Pool engine;
                # the DMA issue rides the (idle) sync queue
                nc.gpsimd.tensor_copy(
                    out=tokbuf[:, lo:hi], in_=idxbuf[:, lo:hi, 0]
                )
                nc.sync.dma_start(out=out[:, lo:hi], in_=tokbuf[:, lo:hi])

            def pair_view(ps):
                # [128, 2, 300] strided view of the two bank halves
                return ps[:].rearrange("p (b q) -> p b q", b=2)[:, :, :K]

            def dst(ps, h):
                return ps[:, h * HALF : h * HALF + K]

            def dve_tile(t, values):
                mx = mxp.tile([P, 8], F32)
                nc.vector.max(out=mx[:], in_=values)
                nc.vector.max_index(
                    out=idxbuf[:, t, :], in_max=mx[:], in_values=values
                )

            ps_pairs = {}

            def alloc_pair(pr):
                ps_pairs[pr] = psp.tile([P, 2 * HALF], F32, name="ps")
                return ps_pairs[pr]

            def bias_write(pr):
                nc.scalar.copy(out=pair_view(alloc_pair(pr)), in_=bft[:])

            def evict_dve(pr):
                val = valp.tile([P, 2, K], F32, name="val")
                nc.scalar.copy(out=val[:], in_=pair_view(ps_pairs[pr]))
                for h in range(2):
                    dve_tile(2 * pr + h, val[:, h, :])

            def jmajor_group(g, bias_mm, mid_scalar=None, post_scalar=None):
                # all 4 tiles' matmuls for chunk j back-to-back, so
                # every arriving 2-chunk batch unlocks ~1us of PE work;
                # evicts inline right after each pair's last matmul
                p0, p1 = 2 * g, 2 * g + 1
                ps0, ps1 = ps_pairs[p0], ps_pairs[p1]
                chunk = chunk_of(g)
                if bias_mm:
                    for tl in range(4):
                        nc.tensor.matmul(
                            dst((ps0, ps1)[tl // 2], tl % 2),
                            lhsT=otile[:], rhs=btile[:],
                            start=True, stop=False,
                            skip_group_check=True,
                        )
                for j in range(DCH):
                    last = j == DCH - 1
                    for tl in (0, 1):
                        nc.tensor.matmul(
                            dst(ps0, tl),
                            lhsT=chunk(j, tl), rhs=ctiles[j][:],
                            start=False, stop=last,
                            skip_group_check=True,
                        )
                    if last:
                        # bias writes go BEFORE the adjacent evict on
                        # the in-order scalar queue: their WAR dep (an
                        # earlier pair's evict) is already done, while
                        # this evict's sem may fire much later
                        if mid_scalar is not None:
                            mid_scalar()
                        evict_dve(p0)
                    for tl in (2, 3):
                        nc.tensor.matmul(
                            dst(ps1, tl - 2),
                            lhsT=chunk(j, tl), rhs=ctiles[j][:],
                            start=False, stop=last,
                            skip_group_check=True,
                        )
                    if last:
                        if post_scalar is not None:
                            post_scalar()
                        evict_dve(p1)

            # ---- prologue: tiles 0-11 (groups 0-2, pairs 0-5)
            # PSUM slot order: pairs 0,1,2,3 -> slots 0-3; the pair
            # 4/5/6 bias writes then land on slots 0/1/2 whose evicts
            # (pairs 0/1/2) are already done when they reach the queue
            # head -- scalar queue: e0 b4 e1 | b5 e2 e3 | b6 e4 e5
            for pr in range(4):
                alloc_pair(pr)
            jmajor_group(0, bias_mm=True, post_scalar=lambda: bias_write(4))
            jmajor_group(1, bias_mm=True, mid_scalar=lambda: bias_write(5))
            jmajor_group(2, bias_mm=False, mid_scalar=lambda: bias_write(6))

            # ---- pairs 6-63: steady state
            pending = None
            for pr in range(2 * JM_GROUPS, PAIRS):
                t0 = 2 * pr
                g = t0 // TPG
                chunk = chunk_of(g)
                if t0 % TPG == 0 and g + 3 < GROUPS and (g + 3) not in gbufs:
                    # prefetch three groups ahead at each group start
                    # (xp bufs=4: active + 3 in flight)
                    issue_group(g + 3)
                ps = ps_pairs[pr]
                if pr + 1 < PAIRS:
                    # bias write for the NEXT pair, one pair ahead of
                    # the PE so the in-order scalar queue never stalls
                    # the matmul stream
                    bias_write(pr + 1)
                # deferred flush: emitted after the next pair's bias
                # copy so the token-DMA issue never parks ahead of an
                # x prefetch issue on the sync queue
                if pending is not None:
                    emit_flush(*pending)
                    pending = None

                for h in range(2):
                    t = t0 + h
                    for j in range(DCH):
                        nc.tensor.matmul(
                            dst(ps, h),
                            lhsT=chunk(j, t % TPG),
                            rhs=ctiles[j][:],
                            start=False,
                            stop=(j == DCH - 1),
                            skip_group_check=True,
                        )

                if pr >= PAIRS - 3:
                    # tail: skip the evict, DVE reads PSUM directly
                    # (+65ns/op vs SBUF but saves the ~0.77us evict
                    # latency, shrinking the end-of-stream DVE backlog)
                    for h in range(2):
                        dve_tile(t0 + h, dst(ps, h))
                else:
                    evict_dve(pr)
                for h in range(2):
                    t = t0 + h
                    if (t + 1) in FLUSH:
                        s = FLUSH[FLUSH.index(t + 1) - 1] if (t + 1) != FLUSH[0] else 0
                        if t + 1 >= TILES - 1:
                            # tail flushes go out immediately: the
                            # 7-tile store overlaps tile 127's argmax,
                            # leaving only a 1-tile store at the end
                            emit_flush(s, t + 1)
                        else:
                            pending = (s, t + 1)

    _hoist_excess_waits(nc)
    return nc


def _hoist_excess_waits(nc: bass.Bass, max_waits: int = 1):
    n = 0
    for f in nc.m.functions:
        for blk in f.blocks:
            insts = blk.instructions
            i = 0
            while i < len(insts):
                inst = insts[i]
                si = inst.sync_info
                if si and si.on_wait and len(si.on_wait) > max_waits:
                    waits = list(si.on_wait)
                    si.on_wait = waits[-max_waits:]
                    inst.sync_info = si
                    pre = []
                    for j in range(0, len(waits) - max_waits, max_waits):
                        nd = mybir.InstNoOp(name=f"I-wsplit{n}", ins=[], outs=[])
                        n += 1
                        nd.engine = inst.engine
                        nsi = type(si)(
                            on_wait=waits[j : j + max_waits], on_update=[]
                        )
                        nd.sync_info = nsi
                        try:
                            nc.register_instruction(nd, overwrite=True)
                        except Exception:
                            pass
                        pre.append(nd)
                    for k, nd in enumerate(pre):
                        insts.insert(i + k, nd)
                    i += len(pre)
                i += 1


def make_in_maps(x, C, Cnorm):
    x16 = x.astype(np.float16)
    C16 = C.astype(np.float16).reshape(DCH, P, K)

    bz = (-0.5 * Cnorm.reshape(K)).astype(np.float32)
    bh = bz.astype(np.float16)
    bl = (bz - bh.astype(np.float32)).astype(np.float16)

    cons = np.zeros((P, DCH * K + K + P), np.float16)
    cons[0, :K] = bh
    cons[1, :K] = bl
    cons[0:2, K : K + P] = 1.0
    cons[:, K + P :] = C16.transpose(1, 0, 2).reshape(P, DCH * K)

    in_maps = []
    for c in range(NCORES):
        xs = x16[c * ROWS : (c + 1) * ROWS]
        xr = xs.reshape(P, GROUPS, TPG, DCH, P)
        xgc = np.ascontiguousarray(xr.transpose(1, 4, 3, 2, 0))
        in_maps.append(
            {
                "xg": xgc.reshape(GROUPS, P, DCH * TPG * P),
                "cons": cons,
            }
        )
    return in_maps


_NC_CACHE = {}


def kernel(x, C, Cnorm, b, t):
    global LAST_RESULT
    x = np.asarray(x)
    C = np.asarray(C)
    Cnorm = np.asarray(Cnorm)

    key = 0
    if key not in _NC_CACHE:
        _NC_CACHE[key] = build_nc()
    nc = _NC_CACHE[key]

    in_maps = make_in_maps(x, C, Cnorm)
    trace = bool(int(os.environ.get("KM_TRACE", "0")))
    if trace:
        _ensure_ntff_hook()
    res = run_bass_kernel_spmd(
        nc, in_maps, core_ids=list(range(NCORES)), trace=trace
    )
    LAST_RESULT = res

    shards = [res.results[c]["out"].reshape(-1) for c in range(NCORES)]
    tokens = np.concatenate(shards).astype(np.int32)
    return tokens.reshape(int(b), int(t))


# revision 12
# speedup vs baseline: 1.0271x; 1.0129x over previous
"""VQ codebook assignment (ApplyKmeans) on 8 Trainium2 NeuronCores.

tokens[n] = argmin_k ||x_n - c_k||^2 = argmax_k (x_n.c_k - Cnorm_k/2)
(||x_n||^2 is constant per row). Data-parallel: x sharded along N
across 8 cores, C/Cnorm replicated.

Per core (16384 rows, 128 row-tiles of 128 rows):
  - host pre-tiles x^T so each [128d, 128n] stationary tile is
    contiguous (fp16: halves HBM traffic; PSUM accumulates fp32;
    52/131072 argmin flips vs the fp32 reference, rel err 0.0144)
  - per tile: 8 accumulating matmuls (x^T chunk stationary, C chunk
    moving) on top of a bias pre-load, so PSUM [128, 300] holds
    val = x.C - Cnorm/2
  - bias pre-load: tiles 0-7 matmul the bias in (ones x [-Cnorm/2
    hi/lo] rows, start=True) which sets every PSUM has_written bit;
    afterwards the otherwise-idle ScalarE rewrites the bank to the
    bias values and the start=False matmuls accumulate onto it -
    saving the PE a 9th matmul per tile
  - VectorE max8 + max_index -> first-occurrence argmax per row
  - dep-free warmup matmuls over a memset tile ramp the PE p-state
    during the startup DMA wait (and their start=True writes cover the
    PSUM has_written bits alongside the tiles-0-7 bias matmuls)
  - tokens compacted on the otherwise-idle Pool engine (DVE is
    co-critical with the PE) and streamed out in 16-tile blocks through
    t=112, then 8/6/2; each flush's emission is deferred one tile so
    the token-DMA issue never parks in the in-order scalar queue ahead
    of the next tile's bias copy

Row interleaving: row-tile t holds rows {p*128 + t}, so the token
buffer [p, t] DMAs out contiguously in original row order.

Walrus only lowers one sync wait per instruction; _hoist_excess_waits
moves Tile's extra waits onto same-engine no-ops at the same program
point. x loads share the sync HWDGE ring (same-ring transfers complete
in order, so prefetch can't starve urgent loads); constants and token
stores ride the scalar ring.

Optimization notes from profiling (kept for future work): steady-state
PE is ~130ns/matmul vs a 127.2 floor; DVE (max8 478 + find_index8 469
+ match 91 per tile) is co-critical with the PE. A pair-FIND variant
(two tiles per 2-bank PSUM tile, one 600-wide FIND) reaches 127ns/MM
steady but loses ~1us net: startup is DMA-bandwidth-bound (~330GB/s
shared across rings, ~2.2us first-transfer latency after the ~7.5us
framework preamble), and this layout''s chunk-major group-0 +
bias-matmul start is better rate-matched to the arrival stream.
Engine clock varies run to run (2.4 vs 2.0 GHz throttle episodes:
check MATMUL median duration ~290ns vs ~348ns in the profile before
comparing timings). A hybrid (this startup + pair-FIND for tiles 8+)
was also tried: with 3-pair PSUM cycling the pair j0 stalls on the
FIND three pairs back (~12us); with singles packed into pair-tile
halves (full 4-pair cycling) it reached steady 129-130ns/MM but still
netted ~159.7us vs this kernel's ~155.5 - the mixed single/pair
pipeline never recovers the pure per-tile schedule's phase.
"""
import os
import sys

import numpy as np

if "/opt/trn_rl_repo" not in sys.path:
    sys.path.insert(0, "/opt/trn_rl_repo")

import concourse.bass as bass
import concourse.mybir as mybir
import concourse.tile_sem_assignment as _tsa
from concourse.bass_utils import run_bass_kernel_spmd
from concourse.tile import TileContext

_tsa.NUM_HWDGE_SEMS = int(os.environ.get("KM_HW_LANES", "8"))

_orig_assign_tick = _tsa.TileClockTick._assign_tick


def _assign_tick_lanepools(self, inst):
    try:
        if isinstance(inst, _tsa.DMAInst) and inst.engine != mybir.EngineType.Pool:
            if not hasattr(self, "_lane_ctr"):
                self._lane_ctr = {}
            eng = inst.engine
            n = _tsa.NUM_HWDGE_SEMS
            half = max(1, n // 2)
            pool = (
                list(range(0, half))
                if eng == mybir.EngineType.Activation
                else list(range(half, n))
            )
            c = self._lane_ctr.get(eng, 0)
            self.next_hw_dma_idx = pool[c % len(pool)]
            self._lane_ctr[eng] = c + 1
    except Exception:
        pass
    return _orig_assign_tick(self, inst)


_tsa.TileClockTick._assign_tick = _assign_tick_lanepools

P = 128
D = 1024
K = 300
NCORES = 8
ROWS = 16384
TILES = ROWS // P
GROUPS = 32
TPG = TILES // GROUPS
DCH = D // P

F16 = mybir.dt.float16
F32 = mybir.dt.float32
I32 = mybir.dt.int32
U32 = mybir.dt.uint32

LAST_RESULT = None


def _ensure_ntff_hook():
    try:
        from antenv.axon_hooks import get_axon_ntff_profile_hook  # noqa: F401

        return
    except ImportError:
        pass
    import types

    import antenv

    try:
        from trn_agent_boot.trn_boot import _ntff_profile_via_ctypes
    except ImportError:
        return
    mod = types.ModuleType("antenv.axon_hooks")
    _hook = [None]
    mod.set_axon_ntff_profile_hook = lambda h: _hook.__setitem__(0, h)
    mod.get_axon_ntff_profile_hook = lambda: _hook[0]
    sys.modules["antenv.axon_hooks"] = mod
    antenv.axon_hooks = mod
    so = "/opt/axon/libaxon_pjrt.so"
    if os.path.exists(so):
        mod.set_axon_ntff_profile_hook(_ntff_profile_via_ctypes(so))


def build_nc(use_act_copy: bool = False) -> bass.Bass:
    nc = bass.Bass()

    xg = nc.declare_dram_parameter("xg", [GROUPS, P, DCH * TPG * P], F16, isOutput=False)
    cons = nc.declare_dram_parameter("cons", [P, DCH * K + K + P], F16, isOutput=False)
    biasf = nc.declare_dram_parameter("biasf", [P, K], F32, isOutput=False)
    out = nc.declare_dram_parameter("out", [P, TILES], I32, isOutput=True)

    FLUSH = [16, 32, 48, 64, 80, 96, 112, 120, 126, 128]

    with TileContext(nc) as tc:
        with (
            tc.tile_pool(name="const", bufs=1) as constp,
            tc.tile_pool(name="warm", bufs=1) as warmp,
            tc.tile_pool(name="xp0", bufs=DCH) as xp0,
            tc.tile_pool(name="xp", bufs=4) as xp,
            tc.tile_pool(name="mx", bufs=8) as mxp,
            tc.tile_pool(name="val", bufs=4) as valp,
            tc.tile_pool(name="psum", bufs=8, space="PSUM") as psp,
            tc.tile_pool(name="outp", bufs=1) as outp,
        ):
            cons_t = constp.tile([P, DCH * K + K + P], F16)
            nc.scalar.dma_start(out=cons_t[:, DCH * K :], in_=cons[:, DCH * K :])
            nc.scalar.dma_start(out=cons_t[:, :K], in_=cons[:, :K])
            nc.scalar.dma_start(out=cons_t[:, K : DCH * K], in_=cons[:, K : DCH * K])
            ctiles = [cons_t[:, j * K : (j + 1) * K] for j in range(DCH)]
            btile = cons_t[:, DCH * K : DCH * K + K]
            otile = cons_t[:, DCH * K + K :]
            bft = constp.tile([P, K], F32)
            nc.scalar.dma_start(out=bft[:], in_=biasf[:])

            # PE warmup: dep-free matmuls over a memset tile, cycling
            # the PSUM banks. They execute during the startup DMA wait
            # (the first const transfer lands ~11.3us after a ~7.5us
            # framework preamble), ramping the PE out of its low
            # p-state so the first real matmuls run at full clock.
            warm = warmp.tile([P, K], F16)
            nc.gpsimd.memset(warm[:], 0.0)
            for w in range(int(os.environ.get("KM_WARM", "8"))):
                wps = psp.tile([P, K], F32, name="ps")
                nc.tensor.matmul(
                    wps[:], lhsT=warm[:, :P], rhs=warm[:],
                    start=True, stop=True,
                )

            xch0 = []
            for j in range(DCH):
                cbuf = xp0.tile([P, TPG, P], F16, name="xchunk")
                nc.sync.dma_start(
                    out=cbuf[:],
                    in_=xg[0, :, j * TPG * P : (j + 1) * TPG * P].rearrange(
                        "p (t q) -> p t q", t=TPG
                    ),
                )
                xch0.append(cbuf)

            idxbuf = outp.tile([P, TILES, 8], U32)
            tokbuf = outp.tile([P, TILES], I32)

            def emit_flush(lo, hi):
                # token compaction on the otherwise-idle Pool engine
                # (it was on DVE, which is co-critical with the PE)
                nc.gpsimd.tensor_copy(
                    out=tokbuf[:, lo:hi], in_=idxbuf[:, lo:hi, 0]
                )
                nc.scalar.dma_start(out=out[:, lo:hi], in_=tokbuf[:, lo:hi])

            pending = None
            for g in range(GROUPS):
                if g == 0:
                    chunk = lambda j, tl: xch0[j][:, tl, :]
                else:
                    xbuf = xp.tile([P, DCH, TPG, P], F16, name="xgrp")
                    nc.sync.dma_start(
                        out=xbuf[:],
                        in_=xg[g].rearrange("p (j t q) -> p j t q", j=DCH, t=TPG),
                    )
                    chunk = lambda j, tl, xbuf=xbuf: xbuf[:, j, tl, :]
                for tl in range(TPG):
                    t = g * TPG + tl
                    psum = psp.tile([P, K], F32, name="ps")
                    if t < 8:
                        nc.tensor.matmul(
                            psum[:], lhsT=otile[:], rhs=btile[:],
                            start=True, stop=False,
                        )
                    else:
                        nc.scalar.copy(out=psum[:], in_=bft[:])
                    # deferred flush: emitted after this tile's bias copy
                    # so the token-DMA issue never parks in the scalar
                    # queue ahead of it (in-order queue would stall the
                    # PE on the previous block's FIND)
                    if pending is not None:
                        emit_flush(*pending)
                        pending = None
                    for j in range(DCH):
                        nc.tensor.matmul(
                            psum[:],
                            lhsT=chunk(j, tl),
                            rhs=ctiles[j][:],
                            start=False,
                            stop=(j == DCH - 1),
                            skip_group_check=(t >= 8),
                        )
                    src = psum
                    mx = mxp.tile([P, 8], F32)
                    nc.vector.max(out=mx[:], in_=src[:])
                    nc.vector.max_index(
                        out=idxbuf[:, t, :], in_max=mx[:], in_values=src[:]
                    )
                    if (t + 1) in FLUSH:
                        s = FLUSH[FLUSH.index(t + 1) - 1] if (t + 1) != FLUSH[0] else 0
                        if t + 1 == TILES:
                            emit_flush(s, t + 1)
                        else:
                            pending = (s, t + 1)

    _hoist_excess_waits(nc)
    return nc


def _hoist_excess_waits(nc: bass.Bass, max_waits: int = 1):
    n = 0
    for f in nc.m.functions:
        for blk in f.blocks:
            insts = blk.instructions
            i = 0
            while i < len(insts):
                inst = insts[i]
                si = inst.sync_info
                if si and si.on_wait and len(si.on_wait) > max_waits:
                    waits = list(si.on_wait)
                    si.on_wait = waits[-max_waits:]
                    inst.sync_info = si
                    pre = []
                    for j in range(0, len(waits) - max_waits, max_waits):
                        nd = mybir.InstNoOp(name=f"I-wsplit{n}", ins=[], outs=[])
                        n += 1
                        nd.engine = inst.engine
                        nsi = type(si)(
                            on_wait=waits[j : j + max_waits], on_update=[]
                        )
                        nd.sync_info = nsi
                        try:
                            nc.register_instruction(nd, overwrite=True)
                        except Exception:
                            pass
                        pre.append(nd)
                    for k, nd in enumerate(pre):
                        insts.insert(i + k, nd)
                    i += len(pre)
                i += 1


def make_in_maps(x, C, Cnorm):
    x16 = x.astype(np.float16)
    C16 = C.astype(np.float16).reshape(DCH, P, K)

    bz = (-0.5 * Cnorm.reshape(K)).astype(np.float32)
    bh = bz.astype(np.float16)
    bl = (bz - bh.astype(np.float32)).astype(np.float16)

    cons = np.zeros((P, DCH * K + K + P), np.float16)
    cons[:, : DCH * K] = C16.transpose(1, 0, 2).reshape(P, DCH * K)
    cons[0, DCH * K : DCH * K + K] = bh
    cons[1, DCH * K : DCH * K + K] = bl
    cons[0:2, DCH * K + K :] = 1.0

    biasf = np.broadcast_to(
        bh.astype(np.float32) + bl.astype(np.float32), (P, K)
    ).copy()

    in_maps = []
    for c in range(NCORES):
        xs = x16[c * ROWS : (c + 1) * ROWS]
        xr = xs.reshape(P, GROUPS, TPG, DCH, P)
        xgc = np.ascontiguousarray(xr.transpose(1, 4, 3, 2, 0))
        in_maps.append(
            {
                "xg": xgc.reshape(GROUPS, P, DCH * TPG * P),
                "cons": cons,
                "biasf": biasf,
            }
        )
    return in_maps


_NC_CACHE = {}


def kernel(x, C, Cnorm, b, t):
    global LAST_RESULT
    x = np.asarray(x)
    C = np.asarray(C)
    Cnorm = np.asarray(Cnorm)

    key = 0
    if key not in _NC_CACHE:
        _NC_CACHE[key] = build_nc()
    nc = _NC_CACHE[key]

    in_maps = make_in_maps(x, C, Cnorm)
    trace = bool(int(os.environ.get("KM_TRACE", "0")))
    if trace:
        _ensure_ntff_hook()
    res = run_bass_kernel_spmd(
        nc, in_maps, core_ids=list(range(NCORES)), trace=trace
    )
    LAST_RESULT = res

    shards = [res.results[c]["out"].reshape(-1) for c in range(NCORES)]
    tokens = np.concatenate(shards).astype(np.int32)
    return tokens.reshape(int(b), int(t))

